# revision 1
# baseline (speedup 1.0000x reference)
"""Builder + host-side sharding for the causal attention head kernel.

B=4, T=2048, C=768, H=64 on 8 NeuronCores, pure data parallel (no
collectives).  Each core owns one batch element and two query quarters
(zigzag pairing for causal load balance): core 2b+0 -> quarters (0, 3),
core 2b+1 -> quarters (1, 2).

Per-core device inputs (host-prearranged, fp16):
  xts   [768, 2048]  x[b].T columns laid out as slots [A | B | F0 | F1]
  wkv   [128, 6*128] [Wk | Wv] pre-tiled partition-major (p, c, m)
  wq    [128, 6*64]  Wq pre-tiled partition-major
  mrows [4, 2048]    rows 0-1: additive causal mask rows (m_a; m_b),
                     rows 2-3: quarter indicator rows (1_a; 1_b)

Device output:
  out   [128, 8*64]  fp32, block blk=quarter*4+half holds rows
                     [blk*128:(blk+1)*128] of the core's [1024, 64] output

The score matrix is computed transposed (s on partitions, q on free dim)
so softmax sums fall out of the PV matmul via a ones-column appended to
V, and the causal mask enters the score matmul as two extra contraction
rows ([K^T; m_a; m_b] against [Q^T; 1_a; 1_b]).  Diagonal 512-blocks get
static triangular multiplicative masks on the exp'd scores.

Schedule: two DMA waves (xts cols 0:1024 = slots A|B, then 1024:2048 =
F0|F1).  Wave-1 projections + diagonal attention run while wave 2
arrives; wave-2 projections interleave with the diagonal groups; the
full (off-diagonal) groups run last at pipeline depth 2.
"""

from contextlib import ExitStack

import numpy as np

import concourse.bass as bass
import concourse.mybir as mybir
import concourse.tile as tile
from concourse import bacc
from concourse.masks import make_identity

FP16 = mybir.dt.float16
F32 = mybir.dt.float32

B, T, C, H = 4, 2048, 768, 64
QTR = 512
N_CORES = 8
SCALE = 1.0 / 8.0  # H ** -0.5
MNEG = -30000.0

# diagonal (tri) groups per quarter: (s_unit pair, mask_sel)
# mask_sel 1 -> [M_0|M_1], 2 -> [M_2|M_3];  M_u[s, q] = 1 iff 128u+s <= q
TRI_GROUPS = [
    [((0, 1), 1), ((2, 3), 2)],        # quarter a: diag units 0..3
    [((4, 5), 1), ((6, 7), 2)],        # quarter b: diag units 4..7
]
# full groups per quarter (no mask)
FULL_GROUPS = [
    [(8, 9), (10, 11)],                         # a: slot F0
    [(0, 1), (2, 3), (8, 9), (10, 11), (12, 13), (14, 15)],  # b: A,F0,F1
]


def build_nc():
    nc = bacc.Bacc("TRN2", target_bir_lowering=False, debug=False,
                   num_devices=N_CORES)
    xts_e = nc.dram_tensor("xts", [C, T], FP16, kind="ExternalInput")
    wkv_e = nc.dram_tensor("wkv", [128, 6 * 128], FP16, kind="ExternalInput")
    wq_e = nc.dram_tensor("wq", [128, 6 * 64], FP16, kind="ExternalInput")
    mrows_e = nc.dram_tensor("mrows", [4, T], FP16, kind="ExternalInput")
    out_e = nc.dram_tensor("out", [128, 8 * 64], F32, kind="ExternalOutput")

    with tile.TileContext(nc) as tc, ExitStack() as ctx:
        ep = ctx.enter_context  # shorthand

        const_p = ep(tc.tile_pool(name="const", bufs=1))
        xt_p = ep(tc.tile_pool(name="xt", bufs=1))
        w_p = ep(tc.tile_pool(name="w", bufs=1))
        big_p = ep(tc.tile_pool(name="big", bufs=1))

        # ---- weights + mask rows first (small, clean 2D DMAs) ----------
        wkv = w_p.tile([128, 6, 128], FP16)
        nc.sync.dma_start(out=wkv[:, :, :],
                          in_=wkv_e[:, :].rearrange("p (n m) -> p n m", m=128))
        wq = w_p.tile([128, 6, 64], FP16)
        nc.scalar.dma_start(out=wq[:, :, :],
                            in_=wq_e[:, :].rearrange("p (n m) -> p n m", m=64))
        kt = big_p.tile([66, T], FP16)        # [K^T ; m_a ; m_b]
        nc.scalar.dma_start(out=kt[64:66, :], in_=mrows_e[0:2, :])
        qt = big_p.tile([66, 2 * QTR], FP16)  # [Q^T ; 1_a 0 ; 0 1_b]
        nc.sync.dma_start(out=qt[64:66, :], in_=mrows_e[2:4, 0:2 * QTR])

        # ---- x^T tiles: one tile, 12 DMAs in two waves, 2 HW queues ----
        xt = xt_p.tile([128, 6, T], FP16)
        for half in range(2):
            hs = slice(half * 1024, (half + 1) * 1024)
            for c in range(6):
                eng = nc.sync if c % 2 == 0 else nc.scalar
                eng.dma_start(out=xt[:, c, hs],
                              in_=xts_e[c * 128:(c + 1) * 128, hs])

        # ---- constants (gpsimd; overlaps DMA) + exp table warm ---------
        ident64 = const_p.tile([128, 64], FP16)
        make_identity(nc, ident64[64:128, :])
        ident65 = const_p.tile([65, 65], FP16)
        make_identity(nc, ident65[:, :])
        mtri = const_p.tile([128, 4 * QTR], FP16)
        nc.gpsimd.memset(mtri[:, :], 1.0)
        warm = const_p.tile([128, 1], FP16)
        nc.scalar.activation(warm[:, :], mtri[:, 0:1],
                             mybir.ActivationFunctionType.Exp, scale=1.0)
        for u in range(4):
            nc.gpsimd.affine_select(
                out=mtri[:, u * QTR:(u + 1) * QTR],
                in_=mtri[:, u * QTR:(u + 1) * QTR],
                compare_op=mybir.AluOpType.is_ge, fill=0.0,
                base=-128 * u, channel_multiplier=-1, pattern=[[1, QTR]],
            )

        vt = big_p.tile([128, T], FP16)       # only rows 64:128 used (V^T)
        v = big_p.tile([128, 16 * 65], FP16)
        v3 = v[:, :].rearrange("p (n m) -> p n m", m=65)
        nc.gpsimd.memset(v3[:, :, 64:65], 1.0)

        ot_ps = ep(tc.tile_pool(name="ot_ps", bufs=1, space="PSUM"))
        exp_p = ep(tc.tile_pool(name="exp", bufs=4))
        o_p = ep(tc.tile_pool(name="o", bufs=2))
        proj_ctx = ExitStack()
        pj_ps = proj_ctx.enter_context(
            tc.tile_pool(name="pj_ps", bufs=1, space="PSUM"))
        tri_ps = proj_ctx.enter_context(
            tc.tile_pool(name="tri_ps", bufs=1, space="PSUM"))

        def proj_alloc(half):
            kv_ps = [pj_ps.tile([128, 512], F32, tag=f"kv{j}",
                                name=f"kv_ps{half}{j}") for j in range(2)]
            q_ps = (pj_ps.tile([128, 512], F32, tag="q", name="q_ps")
                    if half == 0 else None)
            return kv_ps, q_ps

        def proj_c(half, kv_ps, q_ps, c):
            base = half * 1024
            for j in range(2):
                nc.tensor.matmul(
                    kv_ps[j][:, :], wkv[:, c, :],
                    xt[:, c, base + j * 512:base + (j + 1) * 512],
                    start=(c == 0), stop=(c == 5))
            if half == 0:
                for j in range(2):
                    nc.tensor.matmul(
                        q_ps[j * 64:(j + 1) * 64, :], wq[:, c, :],
                        xt[:, c, j * 512:(j + 1) * 512],
                        start=(c == 0), stop=(c == 5),
                        skip_group_check=True)

        def proj_copies(half, kv_ps, q_ps):
            base = half * 1024
            for j in range(2):
                js = slice(base + j * 512, base + (j + 1) * 512)
                nc.vector.tensor_copy(kt[0:64, js], kv_ps[j][0:64, :])
                nc.scalar.copy(vt[64:128, js], kv_ps[j][64:128, :])
            if half == 0:
                for j in range(2):
                    nc.vector.tensor_copy(qt[0:64, j * 512:(j + 1) * 512],
                                          q_ps[j * 64:(j + 1) * 64, :])

        def proj_wave(half):
            kv_ps, q_ps = proj_alloc(half)
            for c in range(6):
                proj_c(half, kv_ps, q_ps, c)
            proj_copies(half, kv_ps, q_ps)

        vtp = tri_ps.tile([128, 16 * 64], FP16)  # packed V transposes, 1 bank

        def v_transposes(half):
            for k in range(8 * half, 8 * half + 8):
                nc.tensor.transpose(vtp[:, k * 64:(k + 1) * 64],
                                    vt[64:128, k * 128:(k + 1) * 128],
                                    ident64[64:128, :])
                nc.vector.tensor_copy(v3[:, k, 0:64],
                                      vtp[:, k * 64:(k + 1) * 64])

        # ---- wave 1: projections for slots A|B (+ Q) -------------------
        proj_wave(0)
        v_transposes(0)

        accs = [ot_ps.tile([65, QTR], F32, tag=f"acc{q}", name=f"acc{q}")
                for q in range(2)]
        n_total = [len(TRI_GROUPS[q]) * 2 + len(FULL_GROUPS[q]) * 2
                   for q in range(2)]
        n_done = [0, 0]

        def attn_group(quarter, units, mask_sel, sg):
            """scores -> exp (-> mask) -> PV for one 2-unit group."""
            qs = slice(quarter * QTR, (quarter + 1) * QTR)
            for i, u in enumerate(units):
                nc.tensor.matmul(sg[:, i * QTR:(i + 1) * QTR],
                                 kt[0:66, u * 128:(u + 1) * 128],
                                 qt[0:66, qs], start=True, stop=True)
            eg = exp_p.tile([128, 2 * QTR], FP16, tag="eg", name="eg")
            nc.scalar.activation(eg[:, :], sg[:, :],
                                 mybir.ActivationFunctionType.Exp,
                                 scale=SCALE)
            if mask_sel:
                ms = slice((mask_sel - 1) * 2 * QTR, mask_sel * 2 * QTR)
                nc.vector.tensor_mul(eg[:, :], eg[:, :], mtri[:, ms])
            acc = accs[quarter]
            for i, u in enumerate(units):
                nc.tensor.matmul(acc[:, :], v3[:, u, :],
                                 eg[:, i * QTR:(i + 1) * QTR],
                                 start=(n_done[quarter] + i == 0),
                                 stop=(n_done[quarter] + i ==
                                       n_total[quarter] - 1))
            n_done[quarter] += 2

        # ---- diagonal groups interleaved with wave-2 projection --------
        # tri phase at pipeline depth 1 (PSUM is full); wave-2 proj MMs
        # fill the PE while exp runs.
        tri_list = [(q, units, m) for q in range(2)
                    for (units, m) in TRI_GROUPS[q]]
        sg1 = tri_ps.tile([128, 2 * QTR], F32)  # single tri scores buffer

        kv2, _ = proj_alloc(1)
        for g, (q, units, m) in enumerate(tri_list):
            qs = slice(q * QTR, (q + 1) * QTR)
            for i, u in enumerate(units):
                nc.tensor.matmul(sg1[:, i * QTR:(i + 1) * QTR],
                                 kt[0:66, u * 128:(u + 1) * 128],
                                 qt[0:66, qs], start=True, stop=True)
            proj_c(1, kv2, None, g)  # wave-2 proj fills PE while exp runs
            eg = exp_p.tile([128, 2 * QTR], FP16, tag="eg", name="eg")
            nc.scalar.activation(eg[:, :], sg1[:, :],
                                 mybir.ActivationFunctionType.Exp,
                                 scale=SCALE)
            ms = slice((m - 1) * 2 * QTR, m * 2 * QTR)
            nc.vector.tensor_mul(eg[:, :], eg[:, :], mtri[:, ms])
            acc = accs[q]
            for i, u in enumerate(units):
                nc.tensor.matmul(acc[:, :], v3[:, u, :],
                                 eg[:, i * QTR:(i + 1) * QTR],
                                 start=(n_done[q] + i == 0), stop=False)
            n_done[q] += 2
        proj_c(1, kv2, None, 4)
        proj_c(1, kv2, None, 5)
        proj_copies(1, kv2, None)
        v_transposes(1)

        # ---- full groups at depth 2, then finalize ---------------------
        proj_ctx.close()
        sc_ps = ep(tc.tile_pool(name="sc_ps", bufs=2, space="PSUM"))
        tr_ps = ep(tc.tile_pool(name="tr_ps", bufs=1, space="PSUM"))
        otp = tr_ps.tile([128, 8 * 68], FP16)  # packed out transposes

        def finalize(quarter):
            acc = accs[quarter]
            ot16 = o_p.tile([65, QTR], FP16, tag="ot16", name="ot16")
            nc.vector.tensor_copy(ot16[:, :], acc[:, :])
            ob = o_p.tile([128, 4 * 64], F32, tag="ob", name="ob")
            for half in range(4):
                blk = quarter * 4 + half
                tp = otp[:, blk * 68:blk * 68 + 65]
                nc.tensor.transpose(tp, ot16[:, half * 128:(half + 1) * 128],
                                    ident65[:, :])
                r = o_p.tile([128, 1], F32, tag="recip", name="recip")
                nc.vector.reciprocal(r[:, :], tp[:, 64:65])
                nc.scalar.activation(ob[:, half * 64:(half + 1) * 64],
                                     tp[:, 0:64],
                                     mybir.ActivationFunctionType.Copy,
                                     scale=r[:, :])
            nc.sync.dma_start(
                out=out_e[:, quarter * 256:(quarter + 1) * 256], in_=ob[:, :])

        full_list = [(q, units) for q in range(2) for units in FULL_GROUPS[q]]
        for idx, (q, units) in enumerate(full_list):
            sg = sc_ps.tile([128, 2 * QTR], F32, tag="sg", name="sg")
            attn_group(q, units, 0, sg)
            if idx == len(FULL_GROUPS[0]) - 1:
                finalize(0)
        finalize(1)

    nc.compile()
    return nc


# ---------------- host-side shard / unshard ---------------------------

def _tile_weights(w, m):
    """[C, m] -> partition-major [128, 6*m] so the DMA is contiguous."""
    return np.ascontiguousarray(
        w.astype(np.float16).reshape(6, 128, m).transpose(1, 0, 2)
        .reshape(128, 6 * m))


def shard_inputs(x, Wq, Wk, Wv):
    """Full inputs -> list of 8 per-core input dicts."""
    wkv = _tile_weights(np.concatenate([Wk, Wv], axis=1), 128)
    wq16 = _tile_weights(Wq, 64)
    zeros = np.zeros(QTR, np.float16)
    negs = np.full(QTR, MNEG, np.float16)
    ones = np.ones(QTR, np.float16)
    in_maps = []
    for b in range(B):
        xT = np.ascontiguousarray(x[b].astype(np.float16).T)  # [C, T]
        q = [xT[:, i * QTR:(i + 1) * QTR] for i in range(4)]
        for h in range(2):
            if h == 0:
                xts = np.concatenate([q[0], q[3], q[1], q[2]], axis=1)
                m_a = np.concatenate([zeros, negs, negs, negs])
                m_b = np.concatenate([zeros, zeros, zeros, zeros])
            else:
                xts = np.concatenate([q[1], q[2], q[0], q[0]], axis=1)
                m_a = np.concatenate([zeros, negs, zeros, negs])
                m_b = np.concatenate([zeros, zeros, zeros, negs])
            ind_a = np.concatenate([ones, zeros, zeros, zeros])
            ind_b = np.concatenate([zeros, ones, zeros, zeros])
            in_maps.append({
                "xts": np.ascontiguousarray(xts),
                "wkv": wkv,
                "wq": wq16,
                "mrows": np.ascontiguousarray(
                    np.stack([m_a, m_b, ind_a, ind_b])),
            })
    return in_maps


def core_out(raw):
    """Device out [128, 8*64] -> the core's [1024, 64] rows."""
    return raw.reshape(128, 8, 64).transpose(1, 0, 2).reshape(1024, 64)


def unshard_outputs(results):
    """List of 8 per-core result dicts -> full [B, T, H] float32."""
    out = np.zeros((B, T, H), dtype=np.float32)
    for b in range(B):
        o0 = core_out(results[2 * b]["out"])
        o1 = core_out(results[2 * b + 1]["out"])
        out[b, 0 * QTR:1 * QTR] = o0[0:QTR]
        out[b, 3 * QTR:4 * QTR] = o0[QTR:2 * QTR]
        out[b, 1 * QTR:2 * QTR] = o1[0:QTR]
        out[b, 2 * QTR:3 * QTR] = o1[QTR:2 * QTR]
    return out


# ---------------- harness entrypoint ----------------------------------

_NC_CACHE = []


def kernel(x, Wq, Wk, Wv):
    """Full inputs -> full [B, T, H] float32 output, computed on 8 cores."""
    from concourse.bass_utils import run_bass_kernel_spmd

    x = np.asarray(x); Wq = np.asarray(Wq)
    Wk = np.asarray(Wk); Wv = np.asarray(Wv)
    in_maps = shard_inputs(x, Wq, Wk, Wv)
    if not _NC_CACHE:
        _NC_CACHE.append(build_nc())
    nc = _NC_CACHE[0]
    res = run_bass_kernel_spmd(nc, in_maps, core_ids=list(range(N_CORES)))
    return unshard_outputs(res.results)

